# revision 20
# baseline (speedup 1.0000x reference)
"""ChessBoardAttention Trainium2 kernel.

Full inputs -> full output. The 32 independent (batch, chessboard-offset)
attention problems are sharded 4-per-core across 8 NeuronCores; the
chessboard gather/scatter is pure data movement done host-side as part of
sharding.

Per-core device kernel, per problem (x_off: [64, 2304]), all matmul
operands bf16:
  qk  = relu(Wqk @ x + b)            [40, L]  one relu per l-block
                                     (q rows 0-7, k rows 32-39);
                                     GpSimd copies k to a base-0 tile
  vT  = relu(x_chunk.T @ Wv.T + bv)  [128-chunks, 65]  col 64 = 1/gamma
  S_T[m, l] = k[:,m-chunk].T @ q     scores TRANSPOSED, 2-m-chunk psum groups
  P_T = exp(S_T)                     split between Act (exact Exp) and DVE
                                     (Schraudolph: bf16 bits = rint(a*s+b)
                                     via fp32->int16 convert, bitcast)
  AV (transposed): out_T[l, c] = sum_m P_T[m, l] vT[m, c] accumulated over
      18 m-chunks into PSUM [128, 65]; col 64 = Z/gamma.
  out_T = (out_T[:, :64] * (gamma/Z)[l]) + xT   fused scalar_tensor_tensor
  Output written l-major [128, 18*64]; host undoes the transpose.

The AV matmuls of block b are interleaved between the score-matmul groups
of block b+1 (and the projection groups of the next problem) so the PE
never idles while the exp engines drain score psum groups.
"""

import numpy as np
import ml_dtypes

import concourse.bass as bass
import concourse.tile as tile
from concourse import mybir
from concourse.bass_utils import run_bass_kernel_spmd

F32 = mybir.dt.float32
BF16 = mybir.dt.bfloat16
I16 = mybir.dt.int16
AT = mybir.AluOpType
AF = mybir.ActivationFunctionType

B, C, H, W = 2, 64, 192, 192
C8 = 8
HQ, WQ = H // 4, W // 4
L = HQ * WQ            # 2304
NPROB = 4              # problems per core
NCORES = 8
NM = L // 128          # 18 m-chunks of 128
LBLOCKS = [(0, 512), (512, 512), (1024, 512), (1536, 512), (2048, 256)]
VS = C + 1             # v-chunk stride in vT_sb (64 channels + 1/gamma col)
SGRP = 3               # m-chunks per score psum group
NGRP = NM // SGRP      # 6 score psum groups per l-block

# Schraudolph exp for bf16: bits16 = rint(A16*s + B16); bitcast int16->bf16.
A16 = float(128.0 / np.log(2.0))
B16 = float(127.0 * 128.0 - 7.4)

# exp engine per score group, cycled per block: Act ~3.67, DVE ~2.33 of 6
EXP_PATTERNS = [
    ["A", "D", "A", "D", "A", "A"],   # 4A/2D
    ["A", "D", "A", "D", "A", "D"],   # 3A/3D
    ["A", "D", "A", "D", "A", "A"],   # 4A/2D
]


def split_drain_waits(nc, keep=1):
    """This walrus build rejects instructions carrying more than a couple of
    sem-waits. Move excess waits onto single-wait DRAIN instructions inserted
    just before the offender on the same engine (drains with one wait are
    known-good through codegen)."""
    for f in nc.m.functions:
        for bb in f.blocks:
            insts = bb.instructions
            idx = 0
            while idx < len(insts):
                i = insts[idx]
                si = i.sync_info
                lim = keep
                if si is not None and si.on_wait and len(si.on_wait) > lim:
                    waits = list(si.on_wait)
                    si.on_wait = waits[-lim:]
                    for k, wt in enumerate(waits[:-lim]):
                        d = mybir.InstDrain(
                            name=f"{i.name}_wsplit{k}", ins=[], outs=[],
                            bass_is_fusable=False,
                        )
                        d.engine = i.engine
                        d.sync_info = mybir.SyncInfo(on_wait=[wt], on_update=[])
                        nc.register_instruction(d)
                        insts.insert(idx, d)
                        idx += 1
                idx += 1


class AvQueue:
    """Pending AV matmuls for one finished l-block, drained a few at a time
    between later PE work so the tensor engine never stalls on exp."""

    def __init__(self, nc, work_pool, small_pool, pT3, st, w, vT3, out_sb,
                 xT_sb, out_dma=None):
        self.nc = nc
        self.small = small_pool
        self.pT3, self.st, self.w = pT3, st, w
        self.vT3, self.out_sb, self.xT_sb = vT3, out_sb, xT_sb
        self.out_dma = out_dma
        self.nsub = w // 128
        self.ps_av = work_pool.tile([128, 512], F32, tag="work")
        self.items = [(sub, mc) for sub in range(self.nsub) for mc in range(NM)]
        self.pos = 0

    def drain(self, n):
        nc = self.nc
        end = min(self.pos + n, len(self.items))
        for i in range(self.pos, end):
            sub, mc = self.items[i]
            nc.tensor.matmul(
                self.ps_av[:, sub * VS : sub * VS + VS],
                lhsT=self.pT3[:, mc, sub * 128 : (sub + 1) * 128],
                rhs=self.vT3[:, mc, :],
                start=(mc == 0), stop=(mc == NM - 1),
            )
        self.pos = end

    def finish(self):
        nc = self.nc
        self.drain(len(self.items))
        # bulk-copy psum->sbuf on DVE, then reciprocal; the per-sub
        # normalize+residual runs on the (otherwise idle) GpSimd engine.
        av_sb = self.small.tile([128, 4 * VS], F32, tag="avc")
        nc.vector.tensor_copy(
            av_sb[:, 0 : self.nsub * VS], self.ps_av[:, 0 : self.nsub * VS])
        rec = self.small.tile([128, 4], F32, tag="rec")
        zview = bass.AP(
            tensor=av_sb.tensor, offset=av_sb.offset + C,
            ap=[list(av_sb.ap)[0], [VS, self.nsub]])
        nc.vector.reciprocal(out=rec[:, 0 : self.nsub], in_=zview)
        for sub in range(self.nsub):
            ci = self.st // 128 + sub
            nc.gpsimd.tensor_scalar_mul(
                av_sb[:, sub * VS : sub * VS + C],
                av_sb[:, sub * VS : sub * VS + C],
                rec[:, sub : sub + 1],
            )
            nc.gpsimd.tensor_tensor(
                out=self.out_sb[:, ci * C : (ci + 1) * C],
                in0=av_sb[:, sub * VS : sub * VS + C],
                in1=self.xT_sb[:, ci * C : (ci + 1) * C],
                op=AT.add,
            )
        if self.out_dma is not None:
            nc.sync.dma_start(out=self.out_dma, in_=self.out_sb)


def build_module():
    nc = bass.Bass("TRN2", target_bir_lowering=False, debug=False,
                   enable_asserts=False)
    xoffs = nc.dram_tensor("xoffs", [NPROB, C, L], BF16, kind="ExternalInput").ap()
    xT_d = nc.dram_tensor("xT", [NPROB, 128, NM * C], F32, kind="ExternalInput").ap()
    wqk = nc.dram_tensor("wqk", [C + 1, 40], BF16, kind="ExternalInput").ap()
    wv = nc.dram_tensor("wv", [C + 1, C], BF16, kind="ExternalInput").ap()
    invg_col = nc.dram_tensor("invg_col", [128, NM], BF16, kind="ExternalInput").ap()
    out_d = nc.dram_tensor("out", [NPROB, 128, NM * C], F32, kind="ExternalOutput").ap()

    with tile.TileContext(nc) as tc:
        with (
            tc.tile_pool(name="singles", bufs=1) as singles,
            tc.tile_pool(name="io", bufs=2) as io,
            tc.tile_pool(name="qk", bufs=2) as qkp,
            tc.tile_pool(name="vt", bufs=2) as vtp,
            tc.tile_pool(name="pt", bufs=2) as ptp,
            tc.tile_pool(name="small", bufs=2) as smallp,
            tc.tile_pool(name="ps_s", bufs=2, space="PSUM") as ps_sp,
            tc.tile_pool(name="work", bufs=2, space="PSUM") as workp,
        ):
            wqk_sb = singles.tile([C + 1, 40], BF16)
            nc.sync.dma_start(out=wqk_sb, in_=wqk)
            wv_sb = singles.tile([C + 1, C], BF16)
            nc.sync.dma_start(out=wv_sb, in_=wv)
            invg_sb = singles.tile([128, NM], BF16)
            nc.sync.dma_start(out=invg_sb, in_=invg_col)

            av_q = None

            def drain(n):
                if av_q is not None:
                    av_q.drain(n)

            def emit_load(p):
                x_sb = io.tile([C + 1, L], BF16, tag="x")
                for st, w in LBLOCKS:
                    nc.sync.dma_start(
                        out=x_sb[0:C, st : st + w], in_=xoffs[p][:, st : st + w])
                nc.gpsimd.memset(x_sb[C : C + 1, :], 1.0)
                xT_sb = io.tile([128, NM * C], F32, tag="xt")
                nc.sync.dma_start(out=xT_sb, in_=xT_d[p])
                out_sb = io.tile([128, NM * C], F32, tag="out")
                return x_sb, xT_sb, out_sb

            def make_proj_tasks(p, x_sb, sink):
                """Projection for problem p as slot-sized tasks. Each task is
                one psum group: a few PE matmuls + one relu (+ k copy)."""
                qk_sb = qkp.tile([40, L], BF16, tag="qk")
                k0_sb = qkp.tile([C8, L], BF16, tag="k0")
                vT_sb = vtp.tile([128, NM * VS], BF16, tag="vt")
                vT3 = vT_sb.rearrange("p (n c) -> p n c", c=VS)
                sink.update(qk=qk_sb, k0=k0_sb, vT3=vT3)

                def qk_task(st, w):
                    def run():
                        ps = workp.tile([128, 512], F32, tag="work")
                        nc.tensor.matmul(
                            ps[:40, :w], lhsT=wqk_sb, rhs=x_sb[:, st : st + w],
                            start=True, stop=True,
                        )
                        nc.vector.tensor_scalar_max(
                            out=qk_sb[:, st : st + w], in0=ps[:40, :w],
                            scalar1=0.0)
                        nc.gpsimd.tensor_copy(
                            k0_sb[:, st : st + w], qk_sb[32:40, st : st + w])
                    return run

                def v_task(g):
                    def run():
                        if g == 0:
                            nc.gpsimd.tensor_copy(vT3[:, :, C], invg_sb)
                        cnt = 8 if g < 2 else NM - 16
                        ps = workp.tile([128, 512], F32, tag="work")
                        for j in range(cnt):
                            mc = g * 8 + j
                            nc.tensor.matmul(
                                ps[:, j * C : (j + 1) * C],
                                lhsT=x_sb[:, mc * 128 : (mc + 1) * 128],
                                rhs=wv_sb, start=True, stop=True,
                            )
                        ps3 = ps.rearrange("p (n c) -> p n c", c=C)
                        nc.vector.tensor_scalar_max(
                            out=vT3[:, g * 8 : g * 8 + cnt, 0:C],
                            in0=ps3[:, 0:cnt, :], scalar1=0.0)
                    return run

                return [qk_task(st, w) for st, w in LBLOCKS] + \
                       [v_task(g) for g in range(3)]

            x_sb, xT_sb, out_sb = emit_load(0)
            sink0 = {}
            for t in make_proj_tasks(0, x_sb, sink0):
                t()
            qk_sb, k0_sb, vT3 = sink0["qk"], sink0["k0"], sink0["vT3"]
            next_load = None
            nsink = {}
            pending = []

            for p in range(NPROB):
                for bi, (st, w) in enumerate(LBLOCKS):
                    if bi == 1 and p + 1 < NPROB:
                        next_load = emit_load(p + 1)
                    if bi == 3 and p + 1 < NPROB:
                        nsink = {}
                        pending = make_proj_tasks(p + 1, next_load[0], nsink)
                    pT_sb = ptp.tile([128, NM * 512], BF16, tag="pt")
                    pT3 = pT_sb.rearrange("p (n c) -> p n c", c=512)
                    eng = EXP_PATTERNS[bi % len(EXP_PATTERNS)]
                    for g in range(NGRP):
                        ps_s = ps_sp.tile([128, SGRP * 512], F32, tag="s")
                        for j in range(SGRP):
                            mc = SGRP * g + j
                            nc.tensor.matmul(
                                ps_s[:, j * 512 : j * 512 + w],
                                lhsT=k0_sb[:, mc * 128 : (mc + 1) * 128],
                                rhs=qk_sb[0:C8, st : st + w],
                                start=True, stop=True,
                            )
                        drain(12)
                        ps_s3 = ps_s.rearrange("p (n c) -> p n c", c=512)
                        if eng[g] == "A":
                            nc.scalar.activation(
                                out=pT3[:, SGRP * g : SGRP * g + SGRP, :w],
                                in_=ps_s3[:, :, :w], func=AF.Exp)
                        else:
                            nc.vector.tensor_scalar(
                                out=pT3[:, SGRP * g : SGRP * g + SGRP, :w]
                                .bitcast(I16),
                                in0=ps_s3[:, :, :w], scalar1=A16, scalar2=B16,
                                op0=AT.mult, op1=AT.add)
                        if pending:
                            pending.pop(0)()
                    if av_q is not None:
                        av_q.finish()
                    is_last = (st, w) == LBLOCKS[-1]
                    av_q = AvQueue(
                        nc, workp, smallp, pT3, st, w, vT3, out_sb, xT_sb,
                        out_dma=out_d[p] if is_last else None)
                if p + 1 < NPROB:
                    x_sb, xT_sb, out_sb = next_load
                    qk_sb, k0_sb, vT3 = nsink["qk"], nsink["k0"], nsink["vT3"]
            av_q.finish()

    split_drain_waits(nc)
    return nc


_NC = None


def _get_nc():
    global _NC
    if _NC is None:
        _NC = build_module()
    return _NC


def make_in_maps(x, Wq, bq, Wk, bk, Wv, bv, gamma):
    bf = ml_dtypes.bfloat16
    x = np.asarray(x, np.float32)
    xoff = (
        x.reshape(B, C, HQ, 4, WQ, 4)
        .transpose(0, 3, 5, 1, 2, 4)
        .reshape(B * 16, C, L)
    )
    xoff_bf = np.ascontiguousarray(xoff.astype(bf))
    # transposed residual, chunk-major: [prob, 128, NM*C]
    xT = np.ascontiguousarray(
        xoff.transpose(0, 2, 1)
        .reshape(B * 16, NM, 128, C)
        .transpose(0, 2, 1, 3)
        .reshape(B * 16, 128, NM * C)
    )
    wqk = np.zeros((C + 1, 40), np.float32)   # q -> psum parts 0-7, k -> 32-39
    wqk[:C, 0:C8] = np.asarray(Wq).T
    wqk[C, 0:C8] = np.asarray(bq)
    wqk[:C, 32:40] = np.asarray(Wk).T
    wqk[C, 32:40] = np.asarray(bk)
    wqk = wqk.astype(bf)
    wv = np.concatenate([np.asarray(Wv).T, np.asarray(bv)[None, :]], 0).astype(bf)
    with np.errstate(divide="ignore"):
        invg = np.float32(1.0) / np.float32(np.asarray(gamma).reshape(-1)[0])
    invg_col = np.full((128, NM), invg, np.float32).astype(bf)
    in_maps = []
    for c in range(NCORES):
        sl = slice(c * NPROB, (c + 1) * NPROB)
        in_maps.append(
            {
                "xoffs": np.ascontiguousarray(xoff_bf[sl]),
                "xT": np.ascontiguousarray(xT[sl]),
                "wqk": wqk,
                "wv": wv,
                "invg_col": invg_col,
            }
        )
    return in_maps


def unshard(results):
    outp = np.concatenate([results[c]["out"] for c in range(NCORES)], 0)
    # [32, 128, NM*C] l-minor-transposed -> [32, C, L]
    outp = (
        outp.reshape(B * 16, 128, NM, C)
        .transpose(0, 3, 2, 1)          # [32, C, NM, 128]
        .reshape(B * 16, C, L)
    )
    return (
        outp.reshape(B, 4, 4, C, HQ, WQ)
        .transpose(0, 3, 4, 1, 5, 2)
        .reshape(B, C, H, W)
        .astype(np.float32)
    )


def kernel(**inputs):
    nc = _get_nc()
    in_maps = make_in_maps(**inputs)
    res = run_bass_kernel_spmd(nc, in_maps, list(range(NCORES)))
    return unshard(res.results)


# revision 25
# speedup vs baseline: 1.1396x; 1.1396x over previous
"""ChessBoardAttention Trainium2 kernel.

Full inputs -> full output. The 32 independent (batch, chessboard-offset)
attention problems are sharded 4-per-core across 8 NeuronCores; the
chessboard gather/scatter is pure data movement done host-side as part of
sharding.

Per-core device kernel, per problem (x_off: [64, 2304]), all matmul
operands bf16:
  qk  = relu(Wqk @ x + b)            [40, L]  one relu per l-block
                                     (q rows 0-7, k rows 32-39);
                                     GpSimd copies k to a base-0 tile
  vT  = relu(x_chunk.T @ Wv.T + bv)  [128-chunks, 65]  col 64 = 1/gamma
  S_T[m, l] = k[:,m-chunk].T @ q     scores TRANSPOSED, 2-m-chunk psum groups
  P_T = exp(S_T)                     split between Act (exact Exp) and DVE
                                     (Schraudolph: bf16 bits = rint(a*s+b)
                                     via fp32->int16 convert, bitcast)
  AV (transposed): out_T[l, c] = sum_m P_T[m, l] vT[m, c] accumulated over
      18 m-chunks into PSUM [128, 65]; col 64 = Z/gamma.
  out_T = (out_T[:, :64] * (gamma/Z)[l]) + xT   fused scalar_tensor_tensor
  Output written l-major [128, 18*64]; host undoes the transpose.

The AV matmuls of block b are interleaved between the score-matmul groups
of block b+1 (and the projection groups of the next problem) so the PE
never idles while the exp engines drain score psum groups.
"""

import numpy as np
import ml_dtypes

import concourse.bass as bass
import concourse.tile as tile
from concourse import mybir
from concourse.bass_utils import run_bass_kernel_spmd

F32 = mybir.dt.float32
BF16 = mybir.dt.bfloat16
I16 = mybir.dt.int16
AT = mybir.AluOpType
AF = mybir.ActivationFunctionType

B, C, H, W = 2, 64, 192, 192
C8 = 8
HQ, WQ = H // 4, W // 4
L = HQ * WQ            # 2304
NPROB = 4              # problems per core
NCORES = 8
NM = L // 128          # 18 m-chunks of 128
LBLOCKS = [(0, 512), (512, 512), (1024, 512), (1536, 512), (2048, 256)]
VS = C + 1             # v-chunk stride in vT_sb (64 channels + 1/gamma col)
SGRP = 2               # m-chunks per score psum group
NGRP = NM // SGRP      # 9 score psum groups per l-block

# Schraudolph exp for bf16: bits16 = rint(A16*s + B16); bitcast int16->bf16.
A16 = float(128.0 / np.log(2.0))
B16 = float(127.0 * 128.0 - 7.4)

# exp engine per score group, cycled per block: Act ~5.3, DVE ~3.7 of 9
EXP_PATTERNS = [
    ["A", "D", "A", "D", "A", "D", "A", "D", "A"],   # 5A/4D
    ["A", "D", "A", "D", "A", "D", "A", "D", "A"],   # 5A/4D
    ["A", "D", "A", "A", "D", "A", "A", "D", "A"],   # 6A/3D
]


def split_drain_waits(nc, keep=1):
    """This walrus build rejects instructions carrying more than a couple of
    sem-waits. Move excess waits onto single-wait DRAIN instructions inserted
    just before the offender on the same engine (drains with one wait are
    known-good through codegen)."""
    for f in nc.m.functions:
        for bb in f.blocks:
            insts = bb.instructions
            idx = 0
            while idx < len(insts):
                i = insts[idx]
                si = i.sync_info
                lim = keep
                if si is not None and si.on_wait and len(si.on_wait) > lim:
                    waits = list(si.on_wait)
                    si.on_wait = waits[-lim:]
                    for k, wt in enumerate(waits[:-lim]):
                        d = mybir.InstDrain(
                            name=f"{i.name}_wsplit{k}", ins=[], outs=[],
                            bass_is_fusable=False,
                        )
                        d.engine = i.engine
                        d.sync_info = mybir.SyncInfo(on_wait=[wt], on_update=[])
                        nc.register_instruction(d)
                        insts.insert(idx, d)
                        idx += 1
                idx += 1


class AvQueue:
    """Pending AV matmuls for one finished l-block, drained a few at a time
    between later PE work so the tensor engine never stalls on exp."""

    def __init__(self, nc, work_pool, small_pool, pT3, st, w, vT3, out_sb,
                 xT_sb, out_dma=None):
        self.nc = nc
        self.small = small_pool
        self.pT3, self.st, self.w = pT3, st, w
        self.vT3, self.out_sb, self.xT_sb = vT3, out_sb, xT_sb
        self.out_dma = out_dma
        self.nsub = w // 128
        self.ps_av = work_pool.tile([128, 512], F32, tag="work")
        self.items = [(sub, mc) for sub in range(self.nsub) for mc in range(NM)]
        self.pos = 0

    def drain(self, n):
        nc = self.nc
        end = min(self.pos + n, len(self.items))
        for i in range(self.pos, end):
            sub, mc = self.items[i]
            nc.tensor.matmul(
                self.ps_av[:, sub * VS : sub * VS + VS],
                lhsT=self.pT3[:, mc, sub * 128 : (sub + 1) * 128],
                rhs=self.vT3[:, mc, :],
                start=(mc == 0), stop=(mc == NM - 1),
            )
        self.pos = end

    def finish(self):
        nc = self.nc
        self.drain(len(self.items))
        rec = self.small.tile([128, 4], F32, tag="rec")
        zview = bass.AP(
            tensor=self.ps_av.tensor, offset=self.ps_av.offset + C,
            ap=[list(self.ps_av.ap)[0], [VS, self.nsub]])
        nc.vector.reciprocal(out=rec[:, 0 : self.nsub], in_=zview)
        for sub in range(self.nsub):
            ci = self.st // 128 + sub
            nc.vector.scalar_tensor_tensor(
                out=self.out_sb[:, ci * C : (ci + 1) * C],
                in0=self.ps_av[:, sub * VS : sub * VS + C],
                scalar=rec[:, sub : sub + 1],
                in1=self.xT_sb[:, ci * C : (ci + 1) * C],
                op0=AT.mult, op1=AT.add,
            )
        if self.out_dma is not None:
            nc.sync.dma_start(out=self.out_dma, in_=self.out_sb)


def build_module():
    nc = bass.Bass("TRN2", target_bir_lowering=False, debug=False,
                   enable_asserts=False)
    xoffs = nc.dram_tensor("xoffs", [NPROB, C, L], BF16, kind="ExternalInput").ap()
    xT_d = nc.dram_tensor("xT", [NPROB, 128, NM * C], F32, kind="ExternalInput").ap()
    wqk = nc.dram_tensor("wqk", [C + 1, 40], BF16, kind="ExternalInput").ap()
    wv = nc.dram_tensor("wv", [C + 1, C], BF16, kind="ExternalInput").ap()
    invg_col = nc.dram_tensor("invg_col", [128, NM], BF16, kind="ExternalInput").ap()
    out_d = nc.dram_tensor("out", [NPROB, 128, NM * C], F32, kind="ExternalOutput").ap()

    with tile.TileContext(nc) as tc:
        with (
            tc.tile_pool(name="singles", bufs=1) as singles,
            tc.tile_pool(name="io", bufs=2) as io,
            tc.tile_pool(name="qk", bufs=2) as qkp,
            tc.tile_pool(name="vt", bufs=2) as vtp,
            tc.tile_pool(name="pt", bufs=2) as ptp,
            tc.tile_pool(name="small", bufs=2) as smallp,
            tc.tile_pool(name="ps_s", bufs=3, space="PSUM") as ps_sp,
            tc.tile_pool(name="work", bufs=2, space="PSUM") as workp,
        ):
            wqk_sb = singles.tile([C + 1, 40], BF16)
            nc.sync.dma_start(out=wqk_sb, in_=wqk)
            wv_sb = singles.tile([C + 1, C], BF16)
            nc.sync.dma_start(out=wv_sb, in_=wv)
            invg_sb = singles.tile([128, NM], BF16)
            nc.sync.dma_start(out=invg_sb, in_=invg_col)

            av_q = None

            def drain(n):
                if av_q is not None:
                    av_q.drain(n)

            def emit_load(p):
                x_sb = io.tile([C + 1, L], BF16, tag="x")
                for st, w in LBLOCKS:
                    nc.sync.dma_start(
                        out=x_sb[0:C, st : st + w], in_=xoffs[p][:, st : st + w])
                nc.gpsimd.memset(x_sb[C : C + 1, :], 1.0)
                xT_sb = io.tile([128, NM * C], F32, tag="xt")
                nc.sync.dma_start(out=xT_sb, in_=xT_d[p])
                out_sb = io.tile([128, NM * C], F32, tag="out")
                return x_sb, xT_sb, out_sb

            def make_proj_tasks(p, x_sb, sink):
                """Projection for problem p as slot-sized tasks. Each task is
                one psum group: a few PE matmuls + one relu (+ k copy)."""
                qk_sb = qkp.tile([40, L], BF16, tag="qk")
                k0_sb = qkp.tile([C8, L], BF16, tag="k0")
                vT_sb = vtp.tile([128, NM * VS], BF16, tag="vt")
                vT3 = vT_sb.rearrange("p (n c) -> p n c", c=VS)
                sink.update(qk=qk_sb, k0=k0_sb, vT3=vT3)

                def qk_task(st, w):
                    def run():
                        ps = workp.tile([128, 512], F32, tag="work")
                        nc.tensor.matmul(
                            ps[:40, :w], lhsT=wqk_sb, rhs=x_sb[:, st : st + w],
                            start=True, stop=True,
                        )
                        nc.scalar.activation(
                            out=qk_sb[:, st : st + w], in_=ps[:40, :w],
                            func=AF.Relu)
                        nc.gpsimd.tensor_copy(
                            k0_sb[:, st : st + w], qk_sb[32:40, st : st + w])
                    return run

                def v_task(g):
                    def run():
                        if g == 0:
                            nc.gpsimd.tensor_copy(vT3[:, :, C], invg_sb)
                        cnt = 8 if g < 2 else NM - 16
                        ps = workp.tile([128, 512], F32, tag="work")
                        for j in range(cnt):
                            mc = g * 8 + j
                            nc.tensor.matmul(
                                ps[:, j * C : (j + 1) * C],
                                lhsT=x_sb[:, mc * 128 : (mc + 1) * 128],
                                rhs=wv_sb, start=True, stop=True,
                            )
                        ps3 = ps.rearrange("p (n c) -> p n c", c=C)
                        nc.vector.tensor_scalar_max(
                            out=vT3[:, g * 8 : g * 8 + cnt, 0:C],
                            in0=ps3[:, 0:cnt, :], scalar1=0.0)
                    return run

                return [qk_task(st, w) for st, w in LBLOCKS] + \
                       [v_task(g) for g in range(3)]

            x_sb, xT_sb, out_sb = emit_load(0)
            sink0 = {}
            for t in make_proj_tasks(0, x_sb, sink0):
                t()
            qk_sb, k0_sb, vT3 = sink0["qk"], sink0["k0"], sink0["vT3"]
            next_load = None
            nsink = {}
            pending = []

            for p in range(NPROB):
                for bi, (st, w) in enumerate(LBLOCKS):
                    if bi == 1 and p + 1 < NPROB:
                        next_load = emit_load(p + 1)
                    if bi == 3 and p + 1 < NPROB:
                        nsink = {}
                        pending = make_proj_tasks(p + 1, next_load[0], nsink)
                    pT_sb = ptp.tile([128, NM * 512], BF16, tag="pt")
                    pT3 = pT_sb.rearrange("p (n c) -> p n c", c=512)
                    eng = EXP_PATTERNS[bi % len(EXP_PATTERNS)]
                    for g in range(NGRP):
                        ps_s = ps_sp.tile([128, SGRP * 512], F32, tag="s")
                        for j in range(SGRP):
                            mc = SGRP * g + j
                            nc.tensor.matmul(
                                ps_s[:, j * 512 : j * 512 + w],
                                lhsT=k0_sb[:, mc * 128 : (mc + 1) * 128],
                                rhs=qk_sb[0:C8, st : st + w],
                                start=True, stop=True,
                            )
                        drain(8)
                        ps_s3 = ps_s.rearrange("p (n c) -> p n c", c=512)
                        if eng[g] == "A":
                            nc.scalar.activation(
                                out=pT3[:, SGRP * g : SGRP * g + SGRP, :w],
                                in_=ps_s3[:, :, :w], func=AF.Exp)
                        else:
                            nc.vector.tensor_scalar(
                                out=pT3[:, SGRP * g : SGRP * g + SGRP, :w]
                                .bitcast(I16),
                                in0=ps_s3[:, :, :w], scalar1=A16, scalar2=B16,
                                op0=AT.mult, op1=AT.add)
                        if pending:
                            pending.pop(0)()
                    if av_q is not None:
                        av_q.finish()
                    is_last = (st, w) == LBLOCKS[-1]
                    av_q = AvQueue(
                        nc, workp, smallp, pT3, st, w, vT3, out_sb, xT_sb,
                        out_dma=out_d[p] if is_last else None)
                if p + 1 < NPROB:
                    x_sb, xT_sb, out_sb = next_load
                    qk_sb, k0_sb, vT3 = nsink["qk"], nsink["k0"], nsink["vT3"]
            av_q.finish()

    split_drain_waits(nc)
    return nc


_NC = None


def _get_nc():
    global _NC
    if _NC is None:
        _NC = build_module()
    return _NC


def make_in_maps(x, Wq, bq, Wk, bk, Wv, bv, gamma):
    bf = ml_dtypes.bfloat16
    x = np.asarray(x, np.float32)
    xoff = (
        x.reshape(B, C, HQ, 4, WQ, 4)
        .transpose(0, 3, 5, 1, 2, 4)
        .reshape(B * 16, C, L)
    )
    xoff_bf = np.ascontiguousarray(xoff.astype(bf))
    # transposed residual, chunk-major: [prob, 128, NM*C]
    xT = np.ascontiguousarray(
        xoff.transpose(0, 2, 1)
        .reshape(B * 16, NM, 128, C)
        .transpose(0, 2, 1, 3)
        .reshape(B * 16, 128, NM * C)
    )
    wqk = np.zeros((C + 1, 40), np.float32)   # q -> psum parts 0-7, k -> 32-39
    wqk[:C, 0:C8] = np.asarray(Wq).T
    wqk[C, 0:C8] = np.asarray(bq)
    wqk[:C, 32:40] = np.asarray(Wk).T
    wqk[C, 32:40] = np.asarray(bk)
    wqk = wqk.astype(bf)
    wv = np.concatenate([np.asarray(Wv).T, np.asarray(bv)[None, :]], 0).astype(bf)
    with np.errstate(divide="ignore"):
        invg = np.float32(1.0) / np.float32(np.asarray(gamma).reshape(-1)[0])
    invg_col = np.full((128, NM), invg, np.float32).astype(bf)
    in_maps = []
    for c in range(NCORES):
        sl = slice(c * NPROB, (c + 1) * NPROB)
        in_maps.append(
            {
                "xoffs": np.ascontiguousarray(xoff_bf[sl]),
                "xT": np.ascontiguousarray(xT[sl]),
                "wqk": wqk,
                "wv": wv,
                "invg_col": invg_col,
            }
        )
    return in_maps


def unshard(results):
    outp = np.concatenate([results[c]["out"] for c in range(NCORES)], 0)
    # [32, 128, NM*C] l-minor-transposed -> [32, C, L]
    outp = (
        outp.reshape(B * 16, 128, NM, C)
        .transpose(0, 3, 2, 1)          # [32, C, NM, 128]
        .reshape(B * 16, C, L)
    )
    return (
        outp.reshape(B, 4, 4, C, HQ, WQ)
        .transpose(0, 3, 4, 1, 5, 2)
        .reshape(B, C, H, W)
        .astype(np.float32)
    )


def kernel(**inputs):
    nc = _get_nc()
    in_maps = make_in_maps(**inputs)
    res = run_bass_kernel_spmd(nc, in_maps, list(range(NCORES)))
    return unshard(res.results)


# revision 26
# speedup vs baseline: 1.1402x; 1.0005x over previous
"""ChessBoardAttention Trainium2 kernel.

Full inputs -> full output. The 32 independent (batch, chessboard-offset)
attention problems are sharded 4-per-core across 8 NeuronCores; the
chessboard gather/scatter is pure data movement done host-side as part of
sharding.

Per-core device kernel, per problem (x_off: [64, 2304]), all matmul
operands bf16:
  qk  = relu(Wqk @ x + b)            [40, L]  one relu per l-block
                                     (q rows 0-7, k rows 32-39);
                                     GpSimd copies k to a base-0 tile
  vT  = relu(x_chunk.T @ Wv.T + bv)  [128-chunks, 65]  col 64 = 1/gamma
  S_T[m, l] = k[:,m-chunk].T @ q     scores TRANSPOSED, 2-m-chunk psum groups
  P_T = exp(S_T)                     split between Act (exact Exp) and DVE
                                     (Schraudolph: bf16 bits = rint(a*s+b)
                                     via fp32->int16 convert, bitcast)
  AV (transposed): out_T[l, c] = sum_m P_T[m, l] vT[m, c] accumulated over
      18 m-chunks into PSUM [128, 65]; col 64 = Z/gamma.
  out_T = (out_T[:, :64] * (gamma/Z)[l]) + xT   fused scalar_tensor_tensor
  Output written l-major [128, 18*64]; host undoes the transpose.

The AV matmuls of block b are interleaved between the score-matmul groups
of block b+1 (and the projection groups of the next problem) so the PE
never idles while the exp engines drain score psum groups.
"""

import numpy as np
import ml_dtypes

import concourse.bass as bass
import concourse.tile as tile
from concourse import mybir
from concourse.bass_utils import run_bass_kernel_spmd

F32 = mybir.dt.float32
BF16 = mybir.dt.bfloat16
I16 = mybir.dt.int16
AT = mybir.AluOpType
AF = mybir.ActivationFunctionType

B, C, H, W = 2, 64, 192, 192
C8 = 8
HQ, WQ = H // 4, W // 4
L = HQ * WQ            # 2304
NPROB = 4              # problems per core
NCORES = 8
NM = L // 128          # 18 m-chunks of 128
LBLOCKS = [(0, 512), (512, 512), (1024, 512), (1536, 512), (2048, 256)]
VS = C + 1             # v-chunk stride in vT_sb (64 channels + 1/gamma col)
SGRP = 2               # m-chunks per score psum group
NGRP = NM // SGRP      # 9 score psum groups per l-block

# Schraudolph exp for bf16: bits16 = rint(A16*s + B16); bitcast int16->bf16.
A16 = float(128.0 / np.log(2.0))
B16 = float(127.0 * 128.0 - 7.4)

# exp engine per score group, cycled per block: Act ~5.3, DVE ~3.7 of 9
EXP_PATTERNS = [
    ["A", "D", "A", "D", "A", "D", "A", "D", "A"],   # 5A/4D
    ["A", "D", "A", "D", "A", "D", "A", "D", "A"],   # 5A/4D
    ["A", "D", "A", "A", "D", "A", "A", "D", "A"],   # 6A/3D
]


def split_drain_waits(nc, keep=1):
    """This walrus build rejects instructions carrying more than a couple of
    sem-waits. Move excess waits onto single-wait DRAIN instructions inserted
    just before the offender on the same engine (drains with one wait are
    known-good through codegen)."""
    for f in nc.m.functions:
        for bb in f.blocks:
            insts = bb.instructions
            idx = 0
            while idx < len(insts):
                i = insts[idx]
                si = i.sync_info
                lim = keep
                if si is not None and si.on_wait and len(si.on_wait) > lim:
                    waits = list(si.on_wait)
                    si.on_wait = waits[-lim:]
                    for k, wt in enumerate(waits[:-lim]):
                        d = mybir.InstDrain(
                            name=f"{i.name}_wsplit{k}", ins=[], outs=[],
                            bass_is_fusable=False,
                        )
                        d.engine = i.engine
                        d.sync_info = mybir.SyncInfo(on_wait=[wt], on_update=[])
                        nc.register_instruction(d)
                        insts.insert(idx, d)
                        idx += 1
                idx += 1


class AvQueue:
    """Pending AV matmuls for one finished l-block, drained a few at a time
    between later PE work so the tensor engine never stalls on exp."""

    def __init__(self, nc, work_pool, small_pool, pT3, st, w, vT3, out_sb,
                 xT_sb, out_dma=None):
        self.nc = nc
        self.small = small_pool
        self.pT3, self.st, self.w = pT3, st, w
        self.vT3, self.out_sb, self.xT_sb = vT3, out_sb, xT_sb
        self.out_dma = out_dma
        self.nsub = w // 128
        self.ps_av = work_pool.tile([128, 512], F32, tag="work")
        self.items = [(sub, mc) for sub in range(self.nsub) for mc in range(NM)]
        self.pos = 0

    def drain(self, n):
        nc = self.nc
        end = min(self.pos + n, len(self.items))
        for i in range(self.pos, end):
            sub, mc = self.items[i]
            nc.tensor.matmul(
                self.ps_av[:, sub * VS : sub * VS + VS],
                lhsT=self.pT3[:, mc, sub * 128 : (sub + 1) * 128],
                rhs=self.vT3[:, mc, :],
                start=(mc == 0), stop=(mc == NM - 1),
            )
        self.pos = end

    def finish(self):
        nc = self.nc
        self.drain(len(self.items))
        rec = self.small.tile([128, 4], F32, tag="rec")
        zview = bass.AP(
            tensor=self.ps_av.tensor, offset=self.ps_av.offset + C,
            ap=[list(self.ps_av.ap)[0], [VS, self.nsub]])
        nc.vector.reciprocal(out=rec[:, 0 : self.nsub], in_=zview)
        for sub in range(self.nsub):
            ci = self.st // 128 + sub
            nc.vector.scalar_tensor_tensor(
                out=self.out_sb[:, ci * C : (ci + 1) * C],
                in0=self.ps_av[:, sub * VS : sub * VS + C],
                scalar=rec[:, sub : sub + 1],
                in1=self.xT_sb[:, ci * C : (ci + 1) * C],
                op0=AT.mult, op1=AT.add,
            )
        if self.out_dma is not None:
            nc.sync.dma_start(out=self.out_dma, in_=self.out_sb)


def build_module():
    nc = bass.Bass("TRN2", target_bir_lowering=False, debug=False,
                   enable_asserts=False)
    xoffs = nc.dram_tensor("xoffs", [NPROB, C, L], BF16, kind="ExternalInput").ap()
    xT_d = nc.dram_tensor("xT", [NPROB, 128, NM * C], F32, kind="ExternalInput").ap()
    wqk = nc.dram_tensor("wqk", [C + 1, 40], BF16, kind="ExternalInput").ap()
    wv = nc.dram_tensor("wv", [C + 1, C], BF16, kind="ExternalInput").ap()
    invg_col = nc.dram_tensor("invg_col", [128, NM], BF16, kind="ExternalInput").ap()
    out_d = nc.dram_tensor("out", [NPROB, 128, NM * C], F32, kind="ExternalOutput").ap()

    with tile.TileContext(nc) as tc:
        with (
            tc.tile_pool(name="singles", bufs=1) as singles,
            tc.tile_pool(name="io", bufs=2) as io,
            tc.tile_pool(name="qk", bufs=2) as qkp,
            tc.tile_pool(name="vt", bufs=2) as vtp,
            tc.tile_pool(name="pt", bufs=3) as ptp,
            tc.tile_pool(name="small", bufs=2) as smallp,
            tc.tile_pool(name="ps_s", bufs=3, space="PSUM") as ps_sp,
            tc.tile_pool(name="work", bufs=2, space="PSUM") as workp,
        ):
            wqk_sb = singles.tile([C + 1, 40], BF16)
            nc.sync.dma_start(out=wqk_sb, in_=wqk)
            wv_sb = singles.tile([C + 1, C], BF16)
            nc.sync.dma_start(out=wv_sb, in_=wv)
            invg_sb = singles.tile([128, NM], BF16)
            nc.sync.dma_start(out=invg_sb, in_=invg_col)

            av_q = None

            def drain(n):
                if av_q is not None:
                    av_q.drain(n)

            def emit_load(p):
                x_sb = io.tile([C + 1, L], BF16, tag="x")
                for st, w in LBLOCKS:
                    nc.sync.dma_start(
                        out=x_sb[0:C, st : st + w], in_=xoffs[p][:, st : st + w])
                nc.gpsimd.memset(x_sb[C : C + 1, :], 1.0)
                xT_sb = io.tile([128, NM * C], F32, tag="xt")
                nc.sync.dma_start(out=xT_sb, in_=xT_d[p])
                out_sb = io.tile([128, NM * C], F32, tag="out")
                return x_sb, xT_sb, out_sb

            def make_proj_tasks(p, x_sb, sink):
                """Projection for problem p as slot-sized tasks. Each task is
                one psum group: a few PE matmuls + one relu (+ k copy)."""
                qk_sb = qkp.tile([40, L], BF16, tag="qk")
                k0_sb = qkp.tile([C8, L], BF16, tag="k0")
                vT_sb = vtp.tile([128, NM * VS], BF16, tag="vt")
                vT3 = vT_sb.rearrange("p (n c) -> p n c", c=VS)
                sink.update(qk=qk_sb, k0=k0_sb, vT3=vT3)

                def qk_task(st, w):
                    def run():
                        ps = workp.tile([128, 512], F32, tag="work")
                        nc.tensor.matmul(
                            ps[:40, :w], lhsT=wqk_sb, rhs=x_sb[:, st : st + w],
                            start=True, stop=True,
                        )
                        nc.scalar.activation(
                            out=qk_sb[:, st : st + w], in_=ps[:40, :w],
                            func=AF.Relu)
                        nc.gpsimd.tensor_copy(
                            k0_sb[:, st : st + w], qk_sb[32:40, st : st + w])
                    return run

                def v_task(g):
                    def run():
                        if g == 0:
                            nc.gpsimd.tensor_copy(vT3[:, :, C], invg_sb)
                        cnt = 8 if g < 2 else NM - 16
                        ps = workp.tile([128, 512], F32, tag="work")
                        for j in range(cnt):
                            mc = g * 8 + j
                            nc.tensor.matmul(
                                ps[:, j * C : (j + 1) * C],
                                lhsT=x_sb[:, mc * 128 : (mc + 1) * 128],
                                rhs=wv_sb, start=True, stop=True,
                            )
                        ps3 = ps.rearrange("p (n c) -> p n c", c=C)
                        nc.vector.tensor_scalar_max(
                            out=vT3[:, g * 8 : g * 8 + cnt, 0:C],
                            in0=ps3[:, 0:cnt, :], scalar1=0.0)
                    return run

                return [qk_task(st, w) for st, w in LBLOCKS] + \
                       [v_task(g) for g in range(3)]

            x_sb, xT_sb, out_sb = emit_load(0)
            sink0 = {}
            for t in make_proj_tasks(0, x_sb, sink0):
                t()
            qk_sb, k0_sb, vT3 = sink0["qk"], sink0["k0"], sink0["vT3"]
            next_load = None
            nsink = {}
            pending = []

            for p in range(NPROB):
                for bi, (st, w) in enumerate(LBLOCKS):
                    if bi == 1 and p + 1 < NPROB:
                        next_load = emit_load(p + 1)
                    if bi == 3 and p + 1 < NPROB:
                        nsink = {}
                        pending = make_proj_tasks(p + 1, next_load[0], nsink)
                    pT_sb = ptp.tile([128, NM * 512], BF16, tag="pt")
                    pT3 = pT_sb.rearrange("p (n c) -> p n c", c=512)
                    eng = EXP_PATTERNS[bi % len(EXP_PATTERNS)]
                    for g in range(NGRP):
                        ps_s = ps_sp.tile([128, SGRP * 512], F32, tag="s")
                        for j in range(SGRP):
                            mc = SGRP * g + j
                            nc.tensor.matmul(
                                ps_s[:, j * 512 : j * 512 + w],
                                lhsT=k0_sb[:, mc * 128 : (mc + 1) * 128],
                                rhs=qk_sb[0:C8, st : st + w],
                                start=True, stop=True,
                            )
                        drain(8)
                        ps_s3 = ps_s.rearrange("p (n c) -> p n c", c=512)
                        if eng[g] == "A":
                            nc.scalar.activation(
                                out=pT3[:, SGRP * g : SGRP * g + SGRP, :w],
                                in_=ps_s3[:, :, :w], func=AF.Exp)
                        else:
                            nc.vector.tensor_scalar(
                                out=pT3[:, SGRP * g : SGRP * g + SGRP, :w]
                                .bitcast(I16),
                                in0=ps_s3[:, :, :w], scalar1=A16, scalar2=B16,
                                op0=AT.mult, op1=AT.add)
                        if pending:
                            pending.pop(0)()
                    if av_q is not None:
                        av_q.finish()
                    is_last = (st, w) == LBLOCKS[-1]
                    av_q = AvQueue(
                        nc, workp, smallp, pT3, st, w, vT3, out_sb, xT_sb,
                        out_dma=out_d[p] if is_last else None)
                if p + 1 < NPROB:
                    x_sb, xT_sb, out_sb = next_load
                    qk_sb, k0_sb, vT3 = nsink["qk"], nsink["k0"], nsink["vT3"]
            av_q.finish()

    split_drain_waits(nc)
    return nc


_NC = None


def _get_nc():
    global _NC
    if _NC is None:
        _NC = build_module()
    return _NC


def make_in_maps(x, Wq, bq, Wk, bk, Wv, bv, gamma):
    bf = ml_dtypes.bfloat16
    x = np.asarray(x, np.float32)
    xoff = (
        x.reshape(B, C, HQ, 4, WQ, 4)
        .transpose(0, 3, 5, 1, 2, 4)
        .reshape(B * 16, C, L)
    )
    xoff_bf = np.ascontiguousarray(xoff.astype(bf))
    # transposed residual, chunk-major: [prob, 128, NM*C]
    xT = np.ascontiguousarray(
        xoff.transpose(0, 2, 1)
        .reshape(B * 16, NM, 128, C)
        .transpose(0, 2, 1, 3)
        .reshape(B * 16, 128, NM * C)
    )
    wqk = np.zeros((C + 1, 40), np.float32)   # q -> psum parts 0-7, k -> 32-39
    wqk[:C, 0:C8] = np.asarray(Wq).T
    wqk[C, 0:C8] = np.asarray(bq)
    wqk[:C, 32:40] = np.asarray(Wk).T
    wqk[C, 32:40] = np.asarray(bk)
    wqk = wqk.astype(bf)
    wv = np.concatenate([np.asarray(Wv).T, np.asarray(bv)[None, :]], 0).astype(bf)
    with np.errstate(divide="ignore"):
        invg = np.float32(1.0) / np.float32(np.asarray(gamma).reshape(-1)[0])
    invg_col = np.full((128, NM), invg, np.float32).astype(bf)
    in_maps = []
    for c in range(NCORES):
        sl = slice(c * NPROB, (c + 1) * NPROB)
        in_maps.append(
            {
                "xoffs": np.ascontiguousarray(xoff_bf[sl]),
                "xT": np.ascontiguousarray(xT[sl]),
                "wqk": wqk,
                "wv": wv,
                "invg_col": invg_col,
            }
        )
    return in_maps


def unshard(results):
    outp = np.concatenate([results[c]["out"] for c in range(NCORES)], 0)
    # [32, 128, NM*C] l-minor-transposed -> [32, C, L]
    outp = (
        outp.reshape(B * 16, 128, NM, C)
        .transpose(0, 3, 2, 1)          # [32, C, NM, 128]
        .reshape(B * 16, C, L)
    )
    return (
        outp.reshape(B, 4, 4, C, HQ, WQ)
        .transpose(0, 3, 4, 1, 5, 2)
        .reshape(B, C, H, W)
        .astype(np.float32)
    )


def kernel(**inputs):
    nc = _get_nc()
    in_maps = make_in_maps(**inputs)
    res = run_bass_kernel_spmd(nc, in_maps, list(range(NCORES)))
    return unshard(res.results)


# revision 28
# speedup vs baseline: 1.1503x; 1.0089x over previous
"""ChessBoardAttention Trainium2 kernel.

Full inputs -> full output. The 32 independent (batch, chessboard-offset)
attention problems are sharded 4-per-core across 8 NeuronCores; the
chessboard gather/scatter is pure data movement done host-side as part of
sharding.

Per-core device kernel, per problem (x_off: [64, 2304]), all matmul
operands bf16:
  qk  = relu(Wqk @ x + b)            [40, L]  one relu per l-block
                                     (q rows 0-7, k rows 32-39);
                                     GpSimd copies k to a base-0 tile
  vT  = relu(x_chunk.T @ Wv.T + bv)  [128-chunks, 65]  col 64 = 1/gamma
  S_T[m, l] = k[:,m-chunk].T @ q     scores TRANSPOSED, 2-m-chunk psum groups
  P_T = exp(S_T)                     split between Act (exact Exp) and DVE
                                     (Schraudolph: bf16 bits = rint(a*s+b)
                                     via fp32->int16 convert, bitcast)
  AV (transposed): out_T[l, c] = sum_m P_T[m, l] vT[m, c] accumulated over
      18 m-chunks into PSUM [128, 65]; col 64 = Z/gamma.
  out_T = (out_T[:, :64] * (gamma/Z)[l]) + xT   fused scalar_tensor_tensor
  Output written l-major [128, 18*64]; host undoes the transpose.

The AV matmuls of block b are interleaved between the score-matmul groups
of block b+1 (and the projection groups of the next problem) so the PE
never idles while the exp engines drain score psum groups.
"""

import numpy as np
import ml_dtypes

import concourse.bass as bass
import concourse.tile as tile
from concourse import mybir
from concourse.bass_utils import run_bass_kernel_spmd

F32 = mybir.dt.float32
BF16 = mybir.dt.bfloat16
I16 = mybir.dt.int16
AT = mybir.AluOpType
AF = mybir.ActivationFunctionType

B, C, H, W = 2, 64, 192, 192
C8 = 8
HQ, WQ = H // 4, W // 4
L = HQ * WQ            # 2304
NPROB = 4              # problems per core
NCORES = 8
NM = L // 128          # 18 m-chunks of 128
LBLOCKS = [(0, 512), (512, 512), (1024, 512), (1536, 512), (2048, 256)]
VS = C + 1             # v-chunk stride in vT_sb (64 channels + 1/gamma col)
SGRP = 2               # m-chunks per score psum group
NGRP = NM // SGRP      # 9 score psum groups per l-block

# Schraudolph exp for bf16: bits16 = rint(A16*s + B16); bitcast int16->bf16.
A16 = float(128.0 / np.log(2.0))
B16 = float(127.0 * 128.0 - 7.4)

# exp engine per score group, cycled per block: Act ~5.3, DVE ~3.7 of 9
EXP_PATTERNS = [
    ["A", "D", "A", "D", "A", "D", "A", "D", "A"],   # 5A/4D
    ["A", "D", "A", "D", "A", "D", "A", "D", "A"],   # 5A/4D
    ["A", "D", "A", "A", "D", "A", "A", "D", "A"],   # 6A/3D
]


def split_drain_waits(nc, keep=1):
    """This walrus build rejects instructions carrying more than a couple of
    sem-waits. Move excess waits onto single-wait DRAIN instructions inserted
    just before the offender on the same engine (drains with one wait are
    known-good through codegen)."""
    for f in nc.m.functions:
        for bb in f.blocks:
            insts = bb.instructions
            idx = 0
            while idx < len(insts):
                i = insts[idx]
                si = i.sync_info
                lim = keep
                if si is not None and si.on_wait and len(si.on_wait) > lim:
                    waits = list(si.on_wait)
                    si.on_wait = waits[-lim:]
                    for k, wt in enumerate(waits[:-lim]):
                        d = mybir.InstDrain(
                            name=f"{i.name}_wsplit{k}", ins=[], outs=[],
                            bass_is_fusable=False,
                        )
                        d.engine = i.engine
                        d.sync_info = mybir.SyncInfo(on_wait=[wt], on_update=[])
                        nc.register_instruction(d)
                        insts.insert(idx, d)
                        idx += 1
                idx += 1


class AvQueue:
    """Pending AV matmuls for one finished l-block, drained a few at a time
    between later PE work so the tensor engine never stalls on exp."""

    def __init__(self, nc, work_pool, small_pool, pT3, st, w, vT3, out_sb,
                 xT_sb, out_dma=None):
        self.nc = nc
        self.small = small_pool
        self.pT3, self.st, self.w = pT3, st, w
        self.vT3, self.out_sb, self.xT_sb = vT3, out_sb, xT_sb
        self.out_dma = out_dma
        self.nsub = w // 128
        self.ps_av = work_pool.tile([128, 512], F32, tag="work")
        self.items = [(sub, mc) for sub in range(self.nsub) for mc in range(NM)]
        self.pos = 0

    def drain(self, n):
        nc = self.nc
        end = min(self.pos + n, len(self.items))
        for i in range(self.pos, end):
            sub, mc = self.items[i]
            nc.tensor.matmul(
                self.ps_av[:, sub * VS : sub * VS + VS],
                lhsT=self.pT3[:, mc, sub * 128 : (sub + 1) * 128],
                rhs=self.vT3[:, mc, :],
                start=(mc == 0), stop=(mc == NM - 1),
            )
        self.pos = end

    def finish(self):
        nc = self.nc
        self.drain(len(self.items))
        rec = self.small.tile([128, 4], F32, tag="rec")
        zview = bass.AP(
            tensor=self.ps_av.tensor, offset=self.ps_av.offset + C,
            ap=[list(self.ps_av.ap)[0], [VS, self.nsub]])
        nc.vector.reciprocal(out=rec[:, 0 : self.nsub], in_=zview)
        for sub in range(self.nsub):
            ci = self.st // 128 + sub
            nc.vector.scalar_tensor_tensor(
                out=self.out_sb[:, ci * C : (ci + 1) * C],
                in0=self.ps_av[:, sub * VS : sub * VS + C],
                scalar=rec[:, sub : sub + 1],
                in1=self.xT_sb[:, ci * C : (ci + 1) * C],
                op0=AT.mult, op1=AT.add,
            )
        if self.out_dma is not None:
            c0 = (self.st // 128) * C
            c1 = c0 + self.nsub * C
            nc.sync.dma_start(
                out=self.out_dma[:, c0:c1], in_=self.out_sb[:, c0:c1])


def build_module():
    nc = bass.Bass("TRN2", target_bir_lowering=False, debug=False,
                   enable_asserts=False)
    xoffs = nc.dram_tensor("xoffs", [NPROB, C, L], BF16, kind="ExternalInput").ap()
    xT_d = nc.dram_tensor("xT", [NPROB, 128, NM * C], F32, kind="ExternalInput").ap()
    wqk = nc.dram_tensor("wqk", [C + 1, 40], BF16, kind="ExternalInput").ap()
    wv = nc.dram_tensor("wv", [C + 1, C], BF16, kind="ExternalInput").ap()
    invg_col = nc.dram_tensor("invg_col", [128, NM], BF16, kind="ExternalInput").ap()
    out_d = nc.dram_tensor("out", [NPROB, 128, NM * C], F32, kind="ExternalOutput").ap()

    with tile.TileContext(nc) as tc:
        with (
            tc.tile_pool(name="singles", bufs=1) as singles,
            tc.tile_pool(name="io", bufs=2) as io,
            tc.tile_pool(name="qk", bufs=2) as qkp,
            tc.tile_pool(name="vt", bufs=2) as vtp,
            tc.tile_pool(name="pt", bufs=3) as ptp,
            tc.tile_pool(name="small", bufs=2) as smallp,
            tc.tile_pool(name="ps_s", bufs=3, space="PSUM") as ps_sp,
            tc.tile_pool(name="work", bufs=2, space="PSUM") as workp,
        ):
            wqk_sb = singles.tile([C + 1, 40], BF16)
            nc.sync.dma_start(out=wqk_sb, in_=wqk)
            wv_sb = singles.tile([C + 1, C], BF16)
            nc.sync.dma_start(out=wv_sb, in_=wv)
            invg_sb = singles.tile([128, NM], BF16)
            nc.sync.dma_start(out=invg_sb, in_=invg_col)

            av_q = None

            def drain(n):
                if av_q is not None:
                    av_q.drain(n)

            def emit_load(p):
                x_sb = io.tile([C + 1, L], BF16, tag="x")
                for st, w in LBLOCKS:
                    nc.sync.dma_start(
                        out=x_sb[0:C, st : st + w], in_=xoffs[p][:, st : st + w])
                nc.gpsimd.memset(x_sb[C : C + 1, :], 1.0)
                xT_sb = io.tile([128, NM * C], F32, tag="xt")
                nc.sync.dma_start(out=xT_sb, in_=xT_d[p])
                out_sb = io.tile([128, NM * C], F32, tag="out")
                return x_sb, xT_sb, out_sb

            def make_proj_tasks(p, x_sb, sink):
                """Projection for problem p as slot-sized tasks. Each task is
                one psum group: a few PE matmuls + one relu (+ k copy)."""
                qk_sb = qkp.tile([40, L], BF16, tag="qk")
                k0_sb = qkp.tile([C8, L], BF16, tag="k0")
                vT_sb = vtp.tile([128, NM * VS], BF16, tag="vt")
                vT3 = vT_sb.rearrange("p (n c) -> p n c", c=VS)
                sink.update(qk=qk_sb, k0=k0_sb, vT3=vT3)

                def qk_task(st, w):
                    def run():
                        ps = workp.tile([128, 512], F32, tag="work")
                        nc.tensor.matmul(
                            ps[:40, :w], lhsT=wqk_sb, rhs=x_sb[:, st : st + w],
                            start=True, stop=True,
                        )
                        nc.scalar.activation(
                            out=qk_sb[:, st : st + w], in_=ps[:40, :w],
                            func=AF.Relu)
                        nc.gpsimd.tensor_copy(
                            k0_sb[:, st : st + w], qk_sb[32:40, st : st + w])
                    return run

                def v_task(g):
                    def run():
                        if g == 0:
                            nc.gpsimd.tensor_copy(vT3[:, :, C], invg_sb)
                        cnt = 8 if g < 2 else NM - 16
                        ps = workp.tile([128, 512], F32, tag="work")
                        for j in range(cnt):
                            mc = g * 8 + j
                            nc.tensor.matmul(
                                ps[:, j * C : (j + 1) * C],
                                lhsT=x_sb[:, mc * 128 : (mc + 1) * 128],
                                rhs=wv_sb, start=True, stop=True,
                            )
                        ps3 = ps.rearrange("p (n c) -> p n c", c=C)
                        nc.vector.tensor_scalar_max(
                            out=vT3[:, g * 8 : g * 8 + cnt, 0:C],
                            in0=ps3[:, 0:cnt, :], scalar1=0.0)
                    return run

                return [qk_task(st, w) for st, w in LBLOCKS] + \
                       [v_task(g) for g in range(3)]

            x_sb, xT_sb, out_sb = emit_load(0)
            sink0 = {}
            for t in make_proj_tasks(0, x_sb, sink0):
                t()
            qk_sb, k0_sb, vT3 = sink0["qk"], sink0["k0"], sink0["vT3"]
            next_load = None
            nsink = {}
            pending = []

            for p in range(NPROB):
                for bi, (st, w) in enumerate(LBLOCKS):
                    if bi == 1 and p + 1 < NPROB:
                        next_load = emit_load(p + 1)
                    if bi == 3 and p + 1 < NPROB:
                        nsink = {}
                        pending = make_proj_tasks(p + 1, next_load[0], nsink)
                    pT_sb = ptp.tile([128, NM * 512], BF16, tag="pt")
                    pT3 = pT_sb.rearrange("p (n c) -> p n c", c=512)
                    eng = EXP_PATTERNS[bi % len(EXP_PATTERNS)]
                    for g in range(NGRP):
                        ps_s = ps_sp.tile([128, SGRP * 512], F32, tag="s")
                        for j in range(SGRP):
                            mc = SGRP * g + j
                            nc.tensor.matmul(
                                ps_s[:, j * 512 : j * 512 + w],
                                lhsT=k0_sb[:, mc * 128 : (mc + 1) * 128],
                                rhs=qk_sb[0:C8, st : st + w],
                                start=True, stop=True,
                            )
                        drain(8)
                        ps_s3 = ps_s.rearrange("p (n c) -> p n c", c=512)
                        if eng[g] == "A":
                            nc.scalar.activation(
                                out=pT3[:, SGRP * g : SGRP * g + SGRP, :w],
                                in_=ps_s3[:, :, :w], func=AF.Exp)
                        else:
                            nc.vector.tensor_scalar(
                                out=pT3[:, SGRP * g : SGRP * g + SGRP, :w]
                                .bitcast(I16),
                                in0=ps_s3[:, :, :w], scalar1=A16, scalar2=B16,
                                op0=AT.mult, op1=AT.add)
                        if pending:
                            pending.pop(0)()
                    if av_q is not None:
                        av_q.finish()
                    is_last = (st, w) == LBLOCKS[-1]
                    av_q = AvQueue(
                        nc, workp, smallp, pT3, st, w, vT3, out_sb, xT_sb,
                        out_dma=out_d[p])
                if p + 1 < NPROB:
                    x_sb, xT_sb, out_sb = next_load
                    qk_sb, k0_sb, vT3 = nsink["qk"], nsink["k0"], nsink["vT3"]
            av_q.finish()

    split_drain_waits(nc)
    return nc


_NC = None


def _get_nc():
    global _NC
    if _NC is None:
        _NC = build_module()
    return _NC


def make_in_maps(x, Wq, bq, Wk, bk, Wv, bv, gamma):
    bf = ml_dtypes.bfloat16
    x = np.asarray(x, np.float32)
    xoff = (
        x.reshape(B, C, HQ, 4, WQ, 4)
        .transpose(0, 3, 5, 1, 2, 4)
        .reshape(B * 16, C, L)
    )
    xoff_bf = np.ascontiguousarray(xoff.astype(bf))
    # transposed residual, chunk-major: [prob, 128, NM*C]
    xT = np.ascontiguousarray(
        xoff.transpose(0, 2, 1)
        .reshape(B * 16, NM, 128, C)
        .transpose(0, 2, 1, 3)
        .reshape(B * 16, 128, NM * C)
    )
    wqk = np.zeros((C + 1, 40), np.float32)   # q -> psum parts 0-7, k -> 32-39
    wqk[:C, 0:C8] = np.asarray(Wq).T
    wqk[C, 0:C8] = np.asarray(bq)
    wqk[:C, 32:40] = np.asarray(Wk).T
    wqk[C, 32:40] = np.asarray(bk)
    wqk = wqk.astype(bf)
    wv = np.concatenate([np.asarray(Wv).T, np.asarray(bv)[None, :]], 0).astype(bf)
    with np.errstate(divide="ignore"):
        invg = np.float32(1.0) / np.float32(np.asarray(gamma).reshape(-1)[0])
    invg_col = np.full((128, NM), invg, np.float32).astype(bf)
    in_maps = []
    for c in range(NCORES):
        sl = slice(c * NPROB, (c + 1) * NPROB)
        in_maps.append(
            {
                "xoffs": np.ascontiguousarray(xoff_bf[sl]),
                "xT": np.ascontiguousarray(xT[sl]),
                "wqk": wqk,
                "wv": wv,
                "invg_col": invg_col,
            }
        )
    return in_maps


def unshard(results):
    outp = np.concatenate([results[c]["out"] for c in range(NCORES)], 0)
    # [32, 128, NM*C] l-minor-transposed -> [32, C, L]
    outp = (
        outp.reshape(B * 16, 128, NM, C)
        .transpose(0, 3, 2, 1)          # [32, C, NM, 128]
        .reshape(B * 16, C, L)
    )
    return (
        outp.reshape(B, 4, 4, C, HQ, WQ)
        .transpose(0, 3, 4, 1, 5, 2)
        .reshape(B, C, H, W)
        .astype(np.float32)
    )


def kernel(**inputs):
    nc = _get_nc()
    in_maps = make_in_maps(**inputs)
    res = run_bass_kernel_spmd(nc, in_maps, list(range(NCORES)))
    return unshard(res.results)
